# revision 11
# baseline (speedup 1.0000x reference)
"""Trainium2 Bass kernel for CDVectorQuantizer eval-mode forward.

Problem: z [32, 256, 4096] f32 (B, D, T), embedding [1024, 256] f32 (K, D).
For each token (b, t): idx = argmin_k ||z[b,:,t] - e_k||^2 ; out[b,:,t] = e_idx.

Math: argmin_k ||z-e_k||^2 == argmax_k (z.e_k - ||e_k||^2/2)  (||z||^2 const per token).

Sharding: data-parallel over batch B across 8 cores (4 batches/core), codebook
replicated. No collectives; host concatenates the per-core outputs.

Per-core kernel (SPMD on 8 cores):
  - Scores via TensorE matmuls in float32r (FP22) using an exact hi/lo split:
    z = z_hi + z_lo, e = e_hi + e_lo with hi = round-to-13-bit-mantissa (FP22
    exact), so z.e = zh.eh + zh.el + zl.eh + (dropped zl.el ~ 1e-8). 3 passes
    at 1 cyc/row instead of fp32's 4 cyc/row, with fp32-level accuracy.
  - Bias add (-||e||^2/2, replicated across partitions) fused with row-max via
    DVE tensor_tensor_reduce; argmax index via DVE max_index.
  - Codebook row gather via GPSIMD indirect DMA from DRAM.
  - [token, d] -> [d, token] layout fix via PE transpose; DMA PSUM->DRAM out.
"""

import numpy as np

import concourse.bacc as bacc
import concourse.bass as bass
import concourse.mybir as mybir
import concourse.tile as tile
from concourse.bass_utils import run_bass_kernel_spmd
from concourse.masks import make_identity

# Problem constants (hardcoded; kernel.py must be self-contained).
B, D, T = 32, 256, 4096
K = 1024
N_CORES = 8
BPC = B // N_CORES  # batches per core
P = 128
DCH = D // P        # 2 contraction chunks of 128
NCH = K // 512      # 2 code chunks of 512 (PSUM bank each)
TCHUNK = 1024       # tokens per z-load chunk
TT = TCHUNK // P    # token tiles per chunk

F32 = mybir.dt.float32
F32R = mybir.dt.float32r
U32 = mybir.dt.uint32
Alu = mybir.AluOpType

# 'f32r3' = 3-pass float32r hi/lo split (fast, ~exact). 'f32' = plain fp32 (slow, exact).
MATMUL_MODE = "f32r3"


def _split(nc, eng, hi_ap, lo_ap, src_ap):
    """hi = src converted to FP22 (engine output rounding for float32r out dtype);
    lo = src - hi, exactly representable in FP22 (small mantissa)."""
    eng.tensor_copy(out=hi_ap, in_=src_ap)
    eng.tensor_tensor(out=lo_ap, in0=src_ap, in1=hi_ap.bitcast(F32), op=Alu.subtract)


def build_vq_kernel():
    nc = bacc.Bacc("TRN2", target_bir_lowering=False, debug=False)
    z = nc.dram_tensor("z", [BPC, D, T], F32, kind="ExternalInput").ap()
    emb = nc.dram_tensor("embedding", [K, D], F32, kind="ExternalInput").ap()
    out = nc.dram_tensor("out", [BPC, D, T], F32, kind="ExternalOutput").ap()

    with tile.TileContext(nc) as tc:
        with tc.tile_pool(name="const", bufs=1) as const:
            identity = const.tile([P, P], F32)
            make_identity(nc, identity[:])
            embT_hi = [const.tile([P, K], F32R, tag=f"embT_hi{c}", name=f"embT_hi{c}") for c in range(DCH)]
            embT_lo = [const.tile([P, K], F32R, tag=f"embT_lo{c}", name=f"embT_lo{c}") for c in range(DCH)]
            embT = [const.tile([P, K], F32, tag=f"embT{c}", name=f"embT{c}") for c in range(DCH)]
            bias_rep = const.tile([P, K], F32)
            bias2 = const.tile([2, K], F32R)
            ones2 = const.tile([2, P], F32R)

            # ---------------- setup: embT, hi/lo split, bias ----------------
            with (
                tc.tile_pool(name="setup", bufs=2) as sp,
                tc.tile_pool(name="setup_ps", bufs=2, space="PSUM") as spp,
            ):
                for j in range(K // P):
                    nat = sp.tile([P, D], F32, tag="nat", bufs=8)
                    nc.sync.dma_start(out=nat[:], in_=emb[j * P : (j + 1) * P, :])
                    for c in range(DCH):
                        tps = spp.tile([P, P], F32, tag="tps")
                        nc.tensor.transpose(
                            out=tps[:],
                            in_=nat[:, c * P : (c + 1) * P],
                            identity=identity[:],
                        )
                        nc.scalar.copy(
                            out=embT[c][:, j * P : (j + 1) * P], in_=tps[:]
                        )
                for c in range(DCH):
                    _split(nc, nc.vector, embT_hi[c][:], embT_lo[c][:], embT[c][:])
                # bias_rep[p, k] = -0.5 * sum_d e[k, d]^2 for every partition p
                reps = []
                for c in range(DCH):
                    sq = sp.tile([P, K], F32, tag="sq")
                    nc.vector.tensor_tensor(
                        out=sq[:], in0=embT[c][:], in1=embT[c][:], op=Alu.mult
                    )
                    rep = sp.tile([P, K], F32, tag=f"rep{c}")
                    nc.gpsimd.partition_all_reduce(
                        rep[:], sq[:], channels=P, reduce_op=bass.bass_isa.ReduceOp.add
                    )
                    reps.append(rep)
                nc.vector.tensor_tensor(
                    out=bias_rep[:], in0=reps[0][:], in1=reps[1][:], op=Alu.add
                )
                nc.vector.tensor_scalar_mul(bias_rep[:], bias_rep[:], -0.5)
                # bias2: [2, K] f32r with row0 = hi(-e2/2), row1 = lo(-e2/2);
                # ones2: [2, P] f32r of ones. ones2.T @ bias2 adds the bias exactly.
                # Compute hi/lo on partition 0 (compute APs must start at p0),
                # then assemble the 2-row tile with SBUF->SBUF DMAs.
                hi0 = sp.tile([1, K], F32R, tag="hi0")
                lo0 = sp.tile([1, K], F32R, tag="lo0")
                nc.vector.tensor_copy(out=hi0[:], in_=bias_rep[0:1, :])
                nc.vector.tensor_tensor(
                    out=lo0[:],
                    in0=bias_rep[0:1, :],
                    in1=hi0[:].bitcast(F32),
                    op=Alu.subtract,
                )
                nc.sync.dma_start(out=bias2[0:1, :], in_=hi0[:])
                nc.sync.dma_start(out=bias2[1:2, :], in_=lo0[:])
                onesf = sp.tile([2, P], F32, tag="onesf")
                nc.gpsimd.memset(onesf[:], 1.0)
                nc.vector.tensor_copy(out=ones2[:], in_=onesf[:])

            # ---------------- main loop ----------------
            with (
                tc.tile_pool(name="zpool", bufs=3) as zp,
                tc.tile_pool(name="spool", bufs=3) as spl,
                tc.tile_pool(name="gpool", bufs=4) as gp,
                tc.tile_pool(name="ps_scores", bufs=2, space="PSUM") as pss,
                tc.tile_pool(name="ps_tr", bufs=2, space="PSUM") as pst,
            ):
                pending = []
                PIPE_DEPTH = 2

                def flush_output(item):
                    gath, fb, ft = item
                    trps = pst.tile([P, D], F32, tag="trps", name="trps")
                    for c in range(DCH):
                        nc.tensor.transpose(
                            out=trps[:, c * P : (c + 1) * P],
                            in_=gath[:, c * P : (c + 1) * P],
                            identity=identity[:],
                        )
                    obuf = gp.tile([P, D], F32, tag="obuf", name="obuf")
                    nc.scalar.copy(out=obuf[:], in_=trps[:])
                    for c in range(DCH):
                        nc.sync.dma_start(
                            out=out[fb, c * P : (c + 1) * P, ft : ft + P],
                            in_=obuf[:, c * P : (c + 1) * P],
                        )

                for b in range(BPC):
                    for t0 in range(0, T, TCHUNK):
                        z_raw = [zp.tile([P, TCHUNK], F32, tag=f"zr{c}", name=f"zr{c}") for c in range(DCH)]
                        z_hi = [zp.tile([P, TCHUNK], F32R, tag=f"zh{c}", name=f"zh{c}") for c in range(DCH)]
                        z_lo = [zp.tile([P, TCHUNK], F32R, tag=f"zl{c}", name=f"zl{c}") for c in range(DCH)]
                        for c in range(DCH):
                            nc.sync.dma_start(
                                out=z_raw[c][:],
                                in_=z[b, c * P : (c + 1) * P, t0 : t0 + TCHUNK],
                            )
                            if MATMUL_MODE == "f32r3":
                                # hi on ScalarE (f32r rounding on write), lo on DVE:
                                # keeps GPSIMD free for indirect gathers and avoids
                                # its slow 2-input ops + ucode library swaps.
                                nc.scalar.copy(out=z_hi[c][:], in_=z_raw[c][:])
                                nc.vector.tensor_tensor(
                                    out=z_lo[c][:],
                                    in0=z_raw[c][:],
                                    in1=z_hi[c][:].bitcast(F32),
                                    op=Alu.subtract,
                                )
                        for tt in range(TT):
                            ts_ = slice(tt * P, (tt + 1) * P)
                            scores_ps = pss.tile([P, K], F32, tag="scores_ps")
                            for n in range(NCH):
                                ns = slice(n * 512, (n + 1) * 512)
                                if MATMUL_MODE == "f32r3":
                                    # distance passes with the C=2 bias matmul mid-group;
                                    # order minimizes stationary reloads.
                                    mms = [
                                        (z_hi[0][:, ts_], embT_hi[0][:, ns]),
                                        (z_hi[0][:, ts_], embT_lo[0][:, ns]),
                                        (ones2[:], bias2[:, ns]),
                                        (z_hi[1][:, ts_], embT_hi[1][:, ns]),
                                        (z_hi[1][:, ts_], embT_lo[1][:, ns]),
                                        (z_lo[0][:, ts_], embT_hi[0][:, ns]),
                                        (z_lo[1][:, ts_], embT_hi[1][:, ns]),
                                    ]
                                    for i, (lt, rt) in enumerate(mms):
                                        nc.tensor.matmul(
                                            out=scores_ps[:, ns],
                                            lhsT=lt,
                                            rhs=rt,
                                            start=(i == 0),
                                            stop=(i == len(mms) - 1),
                                        )
                                else:  # plain fp32
                                    for c in range(DCH):
                                        nc.tensor.matmul(
                                            out=scores_ps[:, ns],
                                            lhsT=z_raw[c][:, ts_],
                                            rhs=embT[c][:, ns],
                                            start=(c == 0),
                                            stop=(c == DCH - 1),
                                        )
                            mx = spl.tile([P, 8], F32, tag="mx")
                            nc.vector.max(out=mx[:], in_=scores_ps[:])
                            idx8 = gp.tile([P, 8], U32, tag="idx")
                            nc.vector.max_index(
                                out=idx8[:], in_max=mx[:], in_values=scores_ps[:]
                            )
                            gath = gp.tile([P, D], F32, tag="gath", bufs=6)
                            nc.gpsimd.indirect_dma_start(
                                out=gath[:],
                                out_offset=None,
                                in_=emb[:],
                                in_offset=bass.IndirectOffsetOnAxis(
                                    ap=idx8[:, 0:1], axis=0
                                ),
                            )
                            # Defer this tile's transpose+writeback a few tiles so
                            # PE never waits on the argmax->gather latency chain.
                            pending.append((gath, b, t0 + tt * P))
                            if len(pending) > PIPE_DEPTH:
                                flush_output(pending.pop(0))
                while pending:
                    flush_output(pending.pop(0))
    nc.compile()
    return nc


_NC_CACHE = None


def _get_nc():
    global _NC_CACHE
    if _NC_CACHE is None:
        _NC_CACHE = build_vq_kernel()
    return _NC_CACHE


def kernel(z: np.ndarray, embedding: np.ndarray, **run_kwargs) -> np.ndarray:
    z = np.ascontiguousarray(np.asarray(z, dtype=np.float32))
    embedding = np.ascontiguousarray(np.asarray(embedding, dtype=np.float32))
    assert z.shape == (B, D, T), z.shape
    assert embedding.shape == (K, D), embedding.shape

    nc = _get_nc()
    in_maps = [
        {"z": z[i * BPC : (i + 1) * BPC], "embedding": embedding}
        for i in range(N_CORES)
    ]
    res = run_bass_kernel_spmd(nc, in_maps, core_ids=list(range(N_CORES)), **run_kwargs)
    out = np.concatenate([r["out"] for r in res.results], axis=0)
    if run_kwargs:
        kernel.last_results = res  # expose profile info to test harness
    return out
